# revision 26
# baseline (speedup 1.0000x reference)
"""Causal self-attention with relative position encoding on 8 Trainium2 NeuronCores.

Problem: B=4, T=1024, C=256, H=8, E=32.
  q,k,v = x@W{q,k,v}.T ; att = q.k + einsum('qjhe,bhqe->bhqj', rel, k) ; scaled,
  causal-masked softmax ; y = att@v ; out = y@Wo.T

Sharding: query-row interleave across 8 cores (core m owns q = m+8t, t in [0,128)).

v2 design (vs baseline):
 - rel pack in fp8(e4m3), pre-scaled x64, consumed with DoubleRow matmuls:
   contraction 256 = 8 q-rows x 32 e per pass -> half DMA bytes, half PE
   stream cycles, half rel matmul count. Q is pre-scaled x64 so content and
   rel share one psum scale; exp uses scale = SCALE/64.
 - content scores: one stream serves all 4 batches via block-diagonal Q4
   stationary [4b*32e, 4b*32tl] against batch-stacked KT4 [4b*32e, j].
 - PV: one matmul per 128-j chunk (V4 stacked [j, 4b*32e] lhsT, transposed-P
   rhs [j, 4b*32tl]); diag 32x32 blocks of the [128,128] psum are the ctx.
 - softmax: no max subtraction (|att*scale| < 1 always), exp w/ accum sums.
 - rel DMA: one 2.2MB dma per head on the sync ring (near peak HBM bw);
   everything else on the scalar ring.
 - per-h khat/Q4 block-diag stationaries built once per head by 4 strided
   copies into pre-zeroed persistent tiles (nonzero slots identical each h).
"""
import os
import numpy as np

import concourse.bass as bass
import concourse.mybir as mybir
import concourse.tile as tile

F32 = mybir.dt.float32
BF16 = mybir.dt.bfloat16
FP8 = mybir.dt.float8e4

B, T, C, H, E = 4, 1024, 256, 8, 32
NC = 8           # cores
TQ = T // NC     # 128 q rows per core
NG = 4           # row groups of 32 q rows
SCALE = 1.0 / np.sqrt(E)
NEG = -1.0e30
RSCL = 64.0      # rel & Q pre-scale
KSCL = 32.0      # K pre-scale (fp8 weight range); exp applies SCALE/(RSCL*KSCL)


def EXT(g, kp2):
    """causal width of rel pass (g, kp2): q-rows tl' in [8*kp2, 8*kp2+8)."""
    return 256 * g + 64 * (kp2 + 1)


PER_G = [2 * (1024 * g + 640) for g in range(NG)]    # fp8 cols per (h,g)
PER_H = sum(PER_G)                                   # 17408
TOTCOL = H * PER_H


def _bloff(g, kp2):
    """col offset of pass (g,kp2) inside a per-h rel block."""
    return sum(PER_G[:g]) + sum(2 * EXT(g, k) for k in range(kp2))


def rel_chunks(ext):
    """split [0,ext) at 512 boundaries (psum bank / moving-dim limits)."""
    if ext <= 512:
        return [(0, ext)]
    return [(0, 512), (512, ext)]


def sanitize_waits(nc):
    """This container's walrus accepts at most ONE sync wait per instruction.
    Hoist extra waits onto same-engine NOPs placed immediately before."""
    n = 0
    for f in nc.m.functions:
        for bb in f.blocks:
            new = []
            for inst in bb.instructions:
                si = inst.sync_info
                if si is not None and si.on_wait and len(si.on_wait) > 1:
                    waits = list(si.on_wait)
                    for w in waits[:-1]:
                        n += 1
                        nop = mybir.InstNoOp(
                            name=f"{inst.name}-sw{n}",
                            engine=inst.engine,
                            sync_info=mybir.SyncInfo(on_wait=[w], on_update=[]),
                            bass_nofuse=True,
                        )
                        new.append(nop)
                    si.on_wait = waits[-1:]
                new.append(inst)
            bb.instructions[:] = new
    return n


def build_program(sanitize=True):
    nc = bass.Bass("TRN2")
    relp_d = nc.dram_tensor("relp", [128, TOTCOL], FP8, kind="ExternalInput")
    xT_d = nc.dram_tensor("xT", [B, C, T], BF16, kind="ExternalInput")
    xT8_d = nc.dram_tensor("xT8", [B, 128, 2 * T], FP8, kind="ExternalInput")
    xqT8_d = nc.dram_tensor("xqT8", [B, 128, 2 * TQ], FP8, kind="ExternalInput")
    Wq8_d = nc.dram_tensor("Wq8", [128, 2 * C], FP8, kind="ExternalInput")
    Wk8_d = nc.dram_tensor("Wk8", [128, 2 * C], FP8, kind="ExternalInput")
    WvT_d = nc.dram_tensor("WvT", [C, C], BF16, kind="ExternalInput")
    WoT_d = nc.dram_tensor("WoT", [C, C], BF16, kind="ExternalInput")
    msk_d = nc.dram_tensor("msk", [128, 256], F32, kind="ExternalInput")
    out_d = nc.dram_tensor("out", [B, TQ, C], F32, kind="ExternalOutput")

    with tile.TileContext(nc) as tc:
        with (
            tc.tile_pool(name="persist", bufs=1) as pp,
            tc.tile_pool(name="stream", bufs=2) as stp,
        ):
            # ---- persistent sbuf tensors ----
            ident = pp.tile([128, 128], BF16, tag="ident", name="ident")
            from concourse.masks import make_identity
            make_identity(nc, ident[:])
            # fp8 DoubleRow projection weights [p, i*256 + c] (i = ci half)
            w8 = {}
            for nm, d in [("Wk8", Wk8_d), ("Wq8", Wq8_d)]:
                t_ = pp.tile([128, 2 * C], FP8, tag=nm, name=nm)
                nc.sync.dma_start(t_[:], d[:, :])
                w8[nm] = t_
            w_sb = {}
            for nm, d in [("WvT", WvT_d), ("WoT", WoT_d)]:
                for half in range(2):
                    t_ = pp.tile([128, 256], BF16, tag=f"{nm}{half}", name=f"{nm}{half}")
                    nc.sync.dma_start(t_[:], d[128 * half:128 * half + 128, :])
                    w_sb[(nm, half)] = t_
            msk = pp.tile([128, 256], F32, tag="msk", name="msk")
            nc.sync.dma_start(msk[:], msk_d[:])
            # K^T at the core's own q columns (khat source, fp8): [c-half, 128b + t]
            kqT = [pp.tile([128, 512], FP8, tag=f"kqT{i}", name=f"kqT{i}") for i in range(2)]
            # Q^T (x64) at core's q columns: [c-half, 128b + t]
            QT = [pp.tile([128, 512], BF16, tag=f"QT{i}", name=f"QT{i}") for i in range(2)]
            # batch-stacked K^T all heads: KT4a[32b+e, 1024h + j] (x32 scale)
            KT4a = pp.tile([128, H * T], BF16, tag="KT4a", name="KT4a")
            # psum-layout staging for the scatter DMA: [32hh+e, 4096ch+1024b+512nk+j]
            KTst = pp.tile([128, H * T], BF16, tag="KTst", name="KTst")
            # batch-stacked V per 128-j block: V4[blk][j, 128h + 32b + e]
            V4 = [pp.tile([128, 1024], BF16, tag=f"V4_{b}", name=f"V4_{b}") for b in range(8)]
            # y^T accumulator: yT[hi][32hh + e, 128b + t]
            yT = [pp.tile([128, 512], BF16, tag=f"yT{i}", name=f"yT{i}") for i in range(2)]
            # block-diagonal stationaries (pre-zeroed once; nonzero slots
            # rewritten per head): khat fp8 [128, 4224], Q4 bf16 [128, 512]
            khat = [pp.tile([128, 4224], FP8, tag=f"khat{i}", name=f"khat{i}") for i in range(3)]
            Q4 = [pp.tile([128, 512], BF16, tag=f"Q4_{i}", name=f"Q4_{i}") for i in range(3)]
            for i in range(3):
                nc.gpsimd.memset(khat[i][:], 0.0)
                nc.gpsimd.memset(Q4[i][:], 0.0)

            # ================= prologue: projections =================
            with tc.tile_pool(name="prjps", bufs=4, space="PSUM") as prjps:
                # --- pass 1: Q^T (x64) and kq^T (x32) for all b (tiny loads,
                # unblocks the khat/Q4 stationary builds early) ---
                for b in range(B):
                    xq8b = stp.tile([128, 2 * TQ], FP8, tag="xq8b", name="xq8b")
                    nc.sync.dma_start(xq8b[:], xqT8_d[b][:, :])
                    for ch in range(2):
                        for ni, (nm, dstT) in enumerate([("Wq8", QT), ("Wk8", kqT)]):
                            ps = prjps.tile([128, 512], F32, tag="prj", name="prj")
                            lhsT = bass.AP(w8[nm][:].tensor, 128 * ch,
                                           [[512, 128], [256, 2], [1, 128]])
                            rhs = bass.AP(xq8b[:].tensor, 0,
                                          [[256, 128], [128, 2], [1, 128]])
                            nc.tensor.matmul(ps[:, 0:TQ], lhsT, rhs, start=True, stop=True,
                                             perf_mode=mybir.MatmulPerfMode.DoubleRow)
                            if ni == 0:
                                nc.scalar.copy(dstT[ch][:, TQ * b:TQ * b + TQ], ps[:, 0:TQ])
                            else:
                                # f32 -> fp8 cast on DVE
                                nc.vector.tensor_copy(dstT[ch][:, TQ * b:TQ * b + TQ], ps[:, 0:TQ])
                # --- pass 2: K (staged for scatter) and V ---
                for b in range(B):
                    xTb = [stp.tile([128, T], BF16, tag=f"xTb{i}", name=f"xTb{i}") for i in range(2)]
                    for i in range(2):
                        nc.sync.dma_start(xTb[i][:], xT_d[b][128 * i:128 * i + 128, :])
                    x8b = stp.tile([128, 2 * T], FP8, tag="x8b", name="x8b")
                    nc.sync.dma_start(x8b[:], xT8_d[b][:, :])
                    # K^T (x32) = Wk8 @ x8 (one DoubleRow matmul per chunk),
                    # staged in psum layout; scattered to KT4a by DMA below
                    for ch in range(2):
                        for nk in range(2):
                            ps = prjps.tile([128, 512], F32, tag="prj", name="prj")
                            lhsT = bass.AP(w8["Wk8"][:].tensor, 128 * ch,
                                           [[512, 128], [256, 2], [1, 128]])
                            rhs = bass.AP(x8b[:].tensor, 512 * nk,
                                          [[2048, 128], [1024, 2], [1, 512]])
                            nc.tensor.matmul(ps[:], lhsT, rhs, start=True, stop=True,
                                             perf_mode=mybir.MatmulPerfMode.DoubleRow)
                            dst = KTst[:, 4096 * ch + 1024 * b + 512 * nk:
                                       4096 * ch + 1024 * b + 512 * nk + 512]
                            if nk == 0:
                                nc.vector.tensor_copy(dst, ps[:])
                            else:
                                nc.scalar.copy(dst, ps[:])
                    # V = x @ Wv^T -> V4 (b,e)-stacked per j-block
                    for blk in range(8):
                        ps = prjps.tile([128, 512], F32, tag="prj", name="prj")
                        for kp in range(2):
                            nc.tensor.matmul(
                                ps[:, 0:256], xTb[kp][:, 128 * blk:128 * blk + 128],
                                w_sb[("WvT", kp)][:], start=(kp == 0), stop=(kp == 1))
                        dst = bass.AP(V4[blk][:].tensor, 32 * b, [[1024, 128], [128, 8], [1, 32]])
                        src = bass.AP(ps[:].tensor, 0, [[512, 128], [32, 8], [1, 32]])
                        if blk % 2 == 0:
                            nc.vector.tensor_copy(dst, src)
                        else:
                            nc.scalar.copy(dst, src)

            # ================= main loop =================
            # software-pipelined over it = (h, g), PV/transpose stage lags
            # scores by SHIFT iterations so PE never waits on the softmax chain
            SHIFT = 3
            with (
                tc.tile_pool(name="rels", bufs=3) as relsp,
                tc.tile_pool(name="pp2", bufs=2 + SHIFT) as pp2,
                tc.tile_pool(name="pts", bufs=3) as ptsp,
                tc.tile_pool(name="stats", bufs=4) as stats,
                tc.tile_pool(name="scps", bufs=2, space="PSUM") as scps,
                tc.tile_pool(name="ptps", bufs=2, space="PSUM") as ptps,
                tc.tile_pool(name="ctxps", bufs=2, space="PSUM") as ctxps,
            ):
                state = {}

                def fetch_rel(h):
                    rels = relsp.tile([128, PER_H], FP8, tag="rels", name="rels")
                    nc.sync.dma_start(rels[:], relp_d[:, PER_H * h:PER_H * (h + 1)])
                    state[(h, 'rels')] = rels

                # prefetch the first two heads' rel blocks, then scatter
                # KTst -> KT4a in head order (content(h) consumes (h, all b))
                fetch_rel(0)
                fetch_rel(1)
                for h in range(H):
                    ch, hh = h // 4, h % 4
                    for b in range(B):
                        c0 = 4096 * ch + 1024 * b
                        nc.sync.dma_start(
                            KT4a[32 * b:32 * b + 32, 1024 * h:1024 * h + 1024],
                            KTst[32 * hh:32 * hh + 32, c0:c0 + 1024])

                def build_stationaries(h):
                    """khat/Q4 block-diag builds for head h (emitted one head
                    ahead so the Pool engine has a full head of lead time)."""
                    hh, hi, hp = h % 4, h // 4, h % 3
                    # khat: k at core's q rows, block-diag fp8 [128, 4224];
                    # nonzero at (32jq+e, 1056g + 256kp2 + 128i + 32b + tl'),
                    # tl' = 8kp2+4i+jq ; src kqT col = 128b + 32g + tl'
                    for jq in range(4):
                        dst = bass.AP(khat[hp][:].tensor, 32 * jq * 4224 + jq,
                                      [[4224, 32], [264, 16], [132, 2], [32, 4]])
                        src = bass.AP(kqT[hi][:].tensor, (32 * hh) * 512 + jq,
                                      [[512, 32], [8, 16], [4, 2], [128, 4]])
                        nc.gpsimd.tensor_copy(dst, src)
                    # Q4: x64 q, block-diag bf16 [128, 512]; nonzero at
                    # (32b+e, 128g + 32b + tl); src QT col = 128b + 32g + tl
                    for b in range(B):
                        dst = bass.AP(Q4[hp][:].tensor, (32 * b) * 512 + 32 * b,
                                      [[512, 32], [128, 4], [1, 32]])
                        src = bass.AP(QT[hi][:].tensor, (32 * hh) * 512 + 128 * b,
                                      [[512, 32], [32, 4], [1, 32]])
                        nc.gpsimd.tensor_copy(dst, src)

                GORD = [3, 2, 1, 0]   # big groups first; tail drains small

                def stage_scores(it):
                    h, gp = it // NG, it % NG
                    g = GORD[gp]
                    hh, hi, hp = h % 4, h // 4, h % 3
                    if gp == 0 and (h, 'rels') not in state:
                        # stream this head's whole rel block (sync ring, ~2.2MB)
                        fetch_rel(h)
                    if gp == 1 and 0 <= h < H - 2:
                        build_stationaries(h + 2)
                    rels = state.pop((h, 'rels')) if gp == NG - 1 else state[(h, 'rels')]
                    eg = 256 * (g + 1)
                    SC = scps.tile([128, 1024], F32, tag="SC", name="SC")
                    # content first (start=True covers [0, eg))
                    for (c0, c1) in rel_chunks(eg):
                        nc.tensor.matmul(SC[:, c0:c1],
                                         Q4[hp][:, 128 * g:128 * g + 128],
                                         KT4a[:, 1024 * h + c0:1024 * h + c1],
                                         start=True, stop=False,
                                         skip_group_check=True)
                    # rel passes: fp8 DoubleRow, contraction 256 = 8 q-rows x 32 e
                    for kp2 in range(4):
                        ext = EXT(g, kp2)
                        bo = _bloff(g, kp2)
                        lhsT = bass.AP(khat[hp][:].tensor, 1056 * g + 256 * kp2,
                                       [[4224, 128], [128, 2], [1, 128]])
                        last = (kp2 == 3)
                        chs = rel_chunks(ext)
                        for ci, (c0, c1) in enumerate(chs):
                            rhs = bass.AP(rels[:].tensor, bo + c0,
                                          [[PER_H, 128], [ext, 2], [1, c1 - c0]])
                            nc.tensor.matmul(SC[:, c0:c1], lhsT, rhs,
                                             start=False,
                                             stop=(last and ci == len(chs) - 1),
                                             perf_mode=mybir.MatmulPerfMode.DoubleRow,
                                             skip_group_check=True)
                    state[it] = SC

                def stage_softmax(it):
                    h, gp = it // NG, it % NG
                    g = GORD[gp]
                    eg = 256 * (g + 1)
                    SC = state.pop(it)
                    nc.vector.tensor_add(SC[:, 256 * g:256 * g + 256],
                                         SC[:, 256 * g:256 * g + 256], msk[:])
                    P = pp2.tile([128, 1024], BF16, tag="P", name="P")
                    sums = stats.tile([128, 1], F32, tag="sums", name="sums")
                    nc.scalar.activation(P[:, 0:eg], SC[:, 0:eg],
                                         mybir.ActivationFunctionType.Exp,
                                         bias=0.0, scale=SCALE / (RSCL * KSCL),
                                         accum_out=sums[:])
                    rec = stats.tile([128, 1], F32, tag="rec", name="rec")
                    nc.vector.reciprocal(rec[:], sums[:])
                    nc.vector.tensor_scalar_mul(P[:, 0:eg], P[:, 0:eg], rec[:])
                    state[(it, 'P')] = P

                def stage_pv(it):
                    h, gp = it // NG, it % NG
                    g = GORD[gp]
                    hh, hi = h % 4, h // 4
                    eg = 256 * (g + 1)
                    P = state.pop((it, 'P'))
                    if gp == 0:
                        # one [128, 512] psum tile accumulates ctx for all 4 g
                        state[(h, 'ctx')] = ctxps.tile([128, 512], F32, tag="ctx", name="ctx")
                    ctx = state[(h, 'ctx')]
                    njb = eg // 128
                    for jj in range(0, njb, 4):
                        nw = min(4, njb - jj)
                        ptp = ptps.tile([128, 512], BF16, tag="PTp", name="PTp")
                        for u in range(nw):
                            nc.tensor.transpose(ptp[:, 128 * u:128 * u + 128],
                                                P[:, 128 * (jj + u):128 * (jj + u) + 128],
                                                ident[:])
                        pts = ptsp.tile([128, 512], BF16, tag="PTs", name="PTs")
                        if (jj // 4) % 2 == 0:
                            nc.vector.tensor_copy(pts[:, 0:128 * nw], ptp[:, 0:128 * nw])
                        else:
                            nc.scalar.copy(pts[:, 0:128 * nw], ptp[:, 0:128 * nw])
                        for u in range(nw):
                            jb = jj + u
                            nc.tensor.matmul(ctx[:, 128 * g:128 * g + 128],
                                             V4[jb][:, 128 * h:128 * h + 128],
                                             pts[:, 128 * u:128 * u + 128],
                                             start=(jb == 0), stop=(jb == njb - 1),
                                             skip_group_check=True)
                    if gp == NG - 1:
                        # diag blocks -> y^T: one [32, 4g x 32tl] copy per batch
                        # src col = 128g' + 32b + tl ; dst col = 128b + 32g' + tl
                        state.pop((h, 'ctx'))
                        for b in range(B):
                            dst = bass.AP(yT[hi][:].tensor,
                                          (32 * hh) * 512 + 128 * b,
                                          [[512, 32], [32, 4], [1, 32]])
                            src = bass.AP(ctx[:].tensor,
                                          (32 * b) * 512 + 32 * b,
                                          [[512, 32], [128, 4], [1, 32]])
                            if b % 2 == 0:
                                nc.vector.tensor_copy(dst, src)
                            else:
                                nc.scalar.copy(dst, src)

                NIT = H * NG
                build_stationaries(0)
                build_stationaries(1)
                for it in range(NIT + SHIFT):
                    if SHIFT <= it:
                        stage_pv(it - SHIFT)
                    if it < NIT:
                        stage_scores(it)
                        stage_softmax(it)

                # ================= output projection =================
                for b in range(B):
                    ps = scps.tile([128, 256], F32, tag="SC", name="SC")
                    for half in range(2):
                        nc.tensor.matmul(ps[:], yT[half][:, 128 * b:128 * b + 128],
                                         w_sb[("WoT", half)][:],
                                         start=(half == 0), stop=(half == 1))
                    ot = pp2.tile([128, 256], F32, tag="oex", name="oex")
                    nc.vector.tensor_copy(ot[:], ps[:])
                    nc.scalar.dma_start(out_d[b][:, :], ot[:])
    if sanitize:
        sanitize_waits(nc)
    return nc


def make_mask(m):
    msk = np.zeros((128, 256), np.float32)
    jj = np.arange(256)[None, :]
    tl = (np.arange(128) % 32)[:, None]
    msk[jj > m + 8 * tl] = NEG
    return msk


def pack_rel_all(rel):
    """fp8 rel packs for all cores: relp_all[m] = [128, TOTCOL].

    Block (h, g, kp2): [128, 2*ext] (i-major), element
    [32jq+e, i*ext + j] = 64*rel[m + 8*(32g + 8kp2 + 4i + jq), j, h, e].
    """
    import ml_dtypes
    r8 = (rel * RSCL).astype(ml_dtypes.float8_e4m3)      # [T, T, H, E]
    # q = 8t + m -> [t, m, j, h, e]
    rr = r8.reshape(TQ, NC, T, H, E)
    relp = np.empty((NC, 128, TOTCOL), ml_dtypes.float8_e4m3)
    for g in range(NG):
        for kp2 in range(4):
            ext = EXT(g, kp2)
            t0 = 32 * g + 8 * kp2
            blk = rr[t0:t0 + 8, :, :ext, :, :]           # [k=4i+jq, m, j, h, e]
            blk = blk.reshape(2, 4, NC, ext, H, E)       # [i, jq, m, j, h, e]
            blk = blk.transpose(2, 4, 1, 5, 0, 3)        # [m, h, jq, e, i, j]
            blk = np.ascontiguousarray(blk).reshape(NC, H, 128, 2 * ext)
            for h in range(H):
                o = PER_H * h + _bloff(g, kp2)
                relp[:, :, o:o + 2 * ext] = blk[:, h]
    return relp


def _dr_pack(a2d):
    """[2*128, N] -> DoubleRow fp8 tile [128, 2*N] (ktile-major cols)."""
    import ml_dtypes
    n = a2d.shape[1]
    return np.ascontiguousarray(
        a2d.astype(ml_dtypes.float8_e4m3).reshape(2, 128, n).transpose(1, 0, 2)
    ).reshape(128, 2 * n)


def host_common(x, Wq, Wk, Wv, Wo):
    import ml_dtypes
    xT = np.ascontiguousarray(x.transpose(0, 2, 1)).astype(ml_dtypes.bfloat16)
    xT8 = np.stack([_dr_pack(xT[b].astype(np.float32)) for b in range(B)])
    return {
        "xT": xT,
        "xT8": xT8,
        "Wq8": _dr_pack(np.asarray(Wq, np.float32).T * RSCL),
        "Wk8": _dr_pack(np.asarray(Wk, np.float32).T * KSCL),
        "WvT": np.ascontiguousarray(np.asarray(Wv, np.float32).T).astype(ml_dtypes.bfloat16),
        "WoT": np.ascontiguousarray(np.asarray(Wo, np.float32).T).astype(ml_dtypes.bfloat16),
    }


def pack_xq8(x, m):
    xq = np.ascontiguousarray(x[:, m::NC, :].transpose(0, 2, 1))  # [B, C, TQ]
    return np.stack([_dr_pack(xq[b]) for b in range(B)])


_CACHE = {}


def kernel(x, rel_encoding, Wq, Wk, Wv, Wo, unused=None, **_):
    x = np.asarray(x, np.float32)
    rel = np.asarray(rel_encoding, np.float32)
    if "ncs" not in _CACHE:
        _CACHE["ncs"] = build_program()
    nc = _CACHE["ncs"]

    import ml_dtypes
    com = host_common(x, Wq, Wk, Wv, Wo)
    relp_all = pack_rel_all(rel)
    in_maps = []
    for m in range(NC):
        im = dict(com)
        im.update({"relp": relp_all[m], "msk": make_mask(m),
                   "xqT8": pack_xq8(x, m)})
        in_maps.append(im)

    from concourse.bass_utils import run_bass_kernel_spmd
    res = run_bass_kernel_spmd(
        nc, in_maps, core_ids=list(range(NC)),
        trace=bool(int(os.environ.get("KERNEL_TRACE", "0"))),
    )
    _CACHE["last_results"] = res
    full = np.empty((B, T, C), np.float32)
    for m in range(NC):
        full[:, m::NC, :] = res.results[m]["out"]
    return full
